# revision 30
# baseline (speedup 1.0000x reference)
"""ConvAConnect Trainium2 kernel — swapped-operand FD=512 variant.

Per-sample noisy conv: Z[b] = conv2d(X[b], W * Werr[b], VALID) + bias * Berr[b].

Data-parallel over batch across 8 NeuronCores (8 samples each). The conv
is 9 shifted matmuls per output block accumulating in PSUM, with the
OPERANDS SWAPPED relative to the classic layout: the stationary is a
[cin=128, cout_half=128] slice of the per-sample noisy filter memW, and
the moving operand is a contiguous [cin=128, 512] spatial slab of X in
64-wide row coordinates. Benefits over the cout-moving layout:
  - moving free dim 512 (vs 256): half the matmul instructions, so the
    fixed ~6-cycle per-matmul overhead halves, and the 384-wide tail
    chunk removes the dead-row padding entirely;
  - the [cin,128] bf16 stationary loads via FWL in ~53ns, deeply hidden
    under the ~216ns moving stream;
  - the per-sample bias lands on PSUM->SBUF move as an ACT-engine
    per-partition bias add (cout now sits on partitions), freeing the DVE
    for the memW muls only.
Output leaves the device as [cout, spatial] bf16; the host transposes,
strips the 2 dead columns per 64-wide row, and upcasts.

Startup discipline (same as the cout-moving variant): the DVFS clock
drops on any PE idle gap and takes ~3.4us of continuous PE busy to reach
full speed, so ACT-memzero-fed warmup matmuls bridge from the earliest
PE dispatch (~8.2us) until sample 0's operands are resident; the DMA
fabric round-robins across outstanding transfers, so phase A (W/Werr
taps 0-2 + X rows 0-9) is fenced ahead of everything else on the queue.
"""

import numpy as np

B, H, Wd, CIN, COUT, KH, KW = 64, 64, 64, 128, 256, 3, 3
HO, WO = H - KH + 1, Wd - KW + 1  # 62, 62
NCORES = 8
S = B // NCORES  # samples per core
SPAT = HO * Wd  # 3968: 62 output rows, 64 wide (2 dead cols per row)
CHUNKS = [(512 * k, 512) for k in range(7)] + [(3584, 384)]  # 7*512+384

PAD = 64  # X tile free-dim pad: tail chunk kh/kw taps read past H*W

TRACE = False  # set by test harness to capture an NTFF profile
LAST_RESULTS = None  # BassKernelResults of the most recent run

_prog_cache = None


def _build_program():
    import concourse.mybir as mybir
    from concourse import bacc
    from concourse.tile import TileContext
    from concourse.tile_rust import add_dep_helper

    f32 = mybir.dt.float32
    bf16 = mybir.dt.bfloat16

    nc = bacc.Bacc()

    X_t = nc.declare_dram_parameter(
        "X_t", [S, CIN, H * Wd + PAD], bf16, isOutput=False
    )
    # W and Werr host-pre-arranged to [cin, (tap cout)]
    W_p = nc.declare_dram_parameter("W", [CIN, KH * KW * COUT], bf16, isOutput=False)
    # bias/Berr host-pre-arranged to [2, 128, 1] / [S, 2, 128, 1] (cout
    # halves onto partitions, trailing 1 so the indexed AP is [128, 1])
    bias_p = nc.declare_dram_parameter("bias", [2, 128, 1], f32, isOutput=False)
    Werr_p = nc.declare_dram_parameter(
        "Werr", [S, CIN, KH * KW * COUT], bf16, isOutput=False
    )
    Berr_p = nc.declare_dram_parameter("Berr", [S, 2, 128, 1], f32, isOutput=False)
    # transposed output: [cout, spatial(62 rows x 64)]; host fixes layout
    OUT = nc.declare_dram_parameter("OUT", [S, COUT, SPAT], bf16, isOutput=True)

    TAPF = KH * KW * COUT  # 2304

    with TileContext(nc) as tc:
        with (
            tc.tile_pool(name="const", bufs=1) as cpool,
            tc.tile_pool(name="xp", bufs=2) as xpool,
            tc.tile_pool(name="wep", bufs=2) as wepool,
            tc.tile_pool(name="mwp", bufs=2) as mwpool,
            tc.tile_pool(name="bbp", bufs=2) as bbpool,
            tc.tile_pool(name="outp", bufs=8) as opool,
            tc.tile_pool(name="ps", bufs=8, space="PSUM") as pspool,
        ):
            warm = cpool.tile([128, 384], bf16)
            nc.scalar.memzero(warm)
            ps_warm = pspool.tile([128, 512], f32, tag="ps")
            NWARM = 38
            for i in range(NWARM):
                nc.tensor.matmul(
                    ps_warm[:, :256],
                    warm[:, :128],
                    warm[:, 128:],
                    start=(i == 0),
                    stop=(i == NWARM - 1),
                )

            W_sb = cpool.tile([CIN, TAPF], bf16)
            W_HEAD = 3 * COUT
            nc.sync.dma_start(out=W_sb[:, :W_HEAD], in_=W_p[:, :W_HEAD])
            # bias halves onto partitions: [128, 2]
            bias_sb = cpool.tile([128, 2], f32)
            nc.gpsimd.dma_start(out=bias_sb[:, 0:1], in_=bias_p[0])
            nc.gpsimd.dma_start(out=bias_sb[:, 1:2], in_=bias_p[1])

            # chunk k (8 output rows) reads X rows 8k..8k+10, so piece 0
            # (rows 0-11) covers chunk 0, piece 1 chunks 1-2, piece 2 the
            # rest (chunk 3 runs ~12us after its piece lands)
            XP0, XP1 = 12 * Wd, 28 * Wd
            s0_last_werr = None
            for s in range(S):
                X_sb = xpool.tile([CIN, H * Wd + PAD], bf16)
                xp0_dma = nc.sync.dma_start(out=X_sb[:, :XP0], in_=X_t[s, :, :XP0])
                if s == 1 and s0_last_werr is not None:
                    add_dep_helper(
                        xp0_dma.ins,
                        s0_last_werr.ins,
                        sync=True,
                        reason="s1 prefetch yields bandwidth to s0 startup",
                    )

                bounds = [0, 1, 3, 6, 9] if s == 0 else [0, 3, 6, 9]
                Werr_sb = wepool.tile([CIN, TAPF], bf16)
                memW = mwpool.tile([CIN, TAPF], bf16)
                for g in range(len(bounds) - 1):
                    lo, hi = bounds[g] * COUT, bounds[g + 1] * COUT
                    if s == 0 and bounds[g] == 3:
                        # phase B opener: W taps 3-8, fenced behind phase A
                        wrest = nc.sync.dma_start(
                            out=W_sb[:, W_HEAD:], in_=W_p[:, W_HEAD:]
                        )
                        add_dep_helper(
                            wrest.ins,
                            xp0_dma.ins,
                            sync=True,
                            reason="phase B yields startup bandwidth to phase A",
                        )
                    wdma = nc.sync.dma_start(
                        out=Werr_sb[:, lo:hi], in_=Werr_p[s, :, lo:hi]
                    )
                    nc.vector.tensor_mul(
                        memW[:, lo:hi], W_sb[:, lo:hi], Werr_sb[:, lo:hi]
                    )
                    if s == 0:
                        s0_last_werr = wdma

                xp1_dma = nc.sync.dma_start(
                    out=X_sb[:, XP0:XP1], in_=X_t[s, :, XP0:XP1]
                )
                if s == 0:
                    # X pieces 1/2 issue right after the Werr tail and would
                    # round-robin against it, delaying the stream start by
                    # ~1.5us; chunk 1 only needs piece 1 at stream+3.9us, so
                    # holding them behind s0's last Werr group is free
                    add_dep_helper(
                        xp1_dma.ins,
                        s0_last_werr.ins,
                        sync=True,
                        reason="s0 X tail yields bandwidth to the Werr tail",
                    )
                nc.sync.dma_start(out=X_sb[:, XP1:], in_=X_t[s, :, XP1:])

                # membias halves on partitions: [128, 2]
                berr_sb = bbpool.tile([128, 2], f32)
                nc.gpsimd.dma_start(out=berr_sb[:, 0:1], in_=Berr_p[s, 0])
                nc.gpsimd.dma_start(out=berr_sb[:, 1:2], in_=Berr_p[s, 1])
                membias = bbpool.tile([128, 2], f32)
                nc.vector.tensor_mul(membias, bias_sb, berr_sb)

                # spatial chunk (base, L) x cout half h: 9 tap matmuls with
                # stationary memW[:, tap*COUT+128h : +128] and moving
                # X_sb[:, base + kh*64 + kw : +L] accumulate [128, L] PSUM
                for base, L in CHUNKS:
                    for h in range(2):
                        ps = pspool.tile([128, 512], f32, tag="ps")
                        mm = 0
                        for kh in range(KH):
                            for kw in range(KW):
                                t = kh * KW + kw
                                lhsT = memW[
                                    :, t * COUT + 128 * h : t * COUT + 128 * h + 128
                                ]
                                rhs = X_sb[:, base + kh * Wd + kw : base + kh * Wd + kw + L]
                                nc.tensor.matmul(
                                    ps[:, :L],
                                    lhsT,
                                    rhs,
                                    start=(mm == 0),
                                    stop=(mm == KH * KW - 1),
                                )
                                mm += 1
                        o_sb = opool.tile([128, 512], bf16)
                        # ACT: psum -> sbuf with per-partition bias add
                        nc.scalar.add(
                            out=o_sb[:, :L], in_=ps[:, :L], add=membias[:, h : h + 1]
                        )
                        nc.scalar.dma_start(
                            out=OUT[s, 128 * h : 128 * (h + 1), base : base + L],
                            in_=o_sb[:, :L],
                        )

    nc.compile()
    return nc


def _get_program():
    global _prog_cache
    if _prog_cache is None:
        _prog_cache = _build_program()
    return _prog_cache


def kernel(X, W, bias, Werr, Berr):
    global LAST_RESULTS
    import ml_dtypes
    from concourse.bass_utils import run_bass_kernel_spmd

    bf16 = ml_dtypes.bfloat16
    X = np.asarray(X, dtype=np.float32)
    W = np.asarray(W, dtype=np.float32)
    bias = np.asarray(bias, dtype=np.float32)
    Werr = np.asarray(Werr, dtype=np.float32)
    Berr = np.asarray(Berr, dtype=np.float32)

    X_t = np.zeros((B, CIN, H * Wd + PAD), bf16)
    X_t[:, :, : H * Wd] = X.transpose(0, 3, 1, 2).reshape(B, CIN, H * Wd).astype(bf16)
    W2 = np.ascontiguousarray(
        W.reshape(KH * KW, CIN, COUT).transpose(1, 0, 2).reshape(CIN, KH * KW * COUT)
    ).astype(bf16)
    Werr2 = np.ascontiguousarray(
        Werr.reshape(B, KH * KW, CIN, COUT)
        .transpose(0, 2, 1, 3)
        .reshape(B, CIN, KH * KW * COUT)
    ).astype(bf16)
    bias2 = np.ascontiguousarray(bias.reshape(2, 128, 1))
    Berr2 = np.ascontiguousarray(Berr.reshape(B, 2, 128, 1))

    nc = _get_program()
    in_maps = []
    for core in range(NCORES):
        sl = slice(core * S, (core + 1) * S)
        in_maps.append(
            {
                "X_t": X_t[sl],
                "W": W2,
                "bias": bias2,
                "Werr": Werr2[sl],
                "Berr": Berr2[sl],
            }
        )

    res = run_bass_kernel_spmd(
        nc, in_maps, core_ids=list(range(NCORES)), trace=TRACE
    )
    LAST_RESULTS = res
    out = np.concatenate([r["OUT"] for r in res.results], axis=0)
    # [B, cout, 62*64] -> [B, 62, 62, cout], strip dead cols, upcast
    out = out.reshape(B, COUT, HO, Wd)[:, :, :, :WO]
    return np.ascontiguousarray(out.transpose(0, 2, 3, 1)).astype(np.float32)
